# revision 23
# baseline (speedup 1.0000x reference)
"""Distributed 2-layer GAT kernel for 8 Trainium2 NeuronCores.

Strategy (host graph preprocessing + device SPMD kernel):
  * Nodes are relabeled by in-degree (ascending) and padded to 20480 ids.
    Blocks of 128 consecutive ids then have near-uniform in-degree, and the
    160 blocks are dealt round-robin to the 8 cores (core = block % 8), so
    every core sees the same per-block degree schedule ghat[l] (compile-time
    constant -> identical SPMD program; all per-core variation is in data).
  * Edge slots are dst-major: slot (block l, k, partition p) holds the k-th
    in-edge of dst p in block l.  A dma_gather pulls the k-th edge row of all
    128 dsts into one [128, cols] SBUF tile (row lands on its dst partition),
    so attention softmax needs no index math on device and the sum over
    in-edges is PSUM accumulation with a constant identity stationary matrix.
  * Each layer's per-node table row = [features bf16 | a_src f32] is built by
    the owning core and AllGather'd (Shared-output HBM scratchpad) so gathers
    are core-local.  AllGather chunks are interleaved with the producer
    blocks so the collective overlaps compute.
  * Pad edge slots point to table row 0, whose a_src is forced to -1e9 on
    host data (alsfix), making exp(leaky_relu(...)) == 0 exactly.
  * Per NAG-chunk 3-pass structure keeps each ACT-table function grouped
    (attention exp + LN stats | batched rstd via exp(-0.5*ln(var+eps)) |
    gelu + next-layer matmul) so activation-table reloads happen per chunk,
    not per block.
"""
import sys

sys.path.insert(0, "/opt/trn_rl_repo")

import numpy as np
import ml_dtypes

from concourse import bass, bacc, tile, mybir
from concourse import bass_utils
from concourse.masks import make_identity

BF16 = ml_dtypes.bfloat16
F32 = mybir.dt.float32
BF = mybir.dt.bfloat16
I16 = mybir.dt.int16
AF = mybir.ActivationFunctionType
OP = mybir.AluOpType

# problem constants
N, E = 20000, 320000
D_IN, HID, D_OUT = 128, 128, 32
H1, H2 = 4, 1
EPS = 1e-5

NCORES = 8
P = 128
NPAD = 20480            # padded node count: 160 blocks of 128
NBLK_G = NPAD // P      # 160 global blocks
NPB = NPAD // NCORES    # 2560 nodes per core
NBLK = NPB // P         # 20 blocks per core
NEG = -1e9

T1COLS = 640            # L1 table row (bf16): 512 feats | 4 f32 a_src | pad
T1USED = 520
T2COLS = 256            # L2 table row (bf16): 128 feats | 1 f32 a_src | pad
T2USED = 130
KC = 8                  # max in-edge slots per gather call
NAG = 4                 # AllGather chunks (interleaved with producers)
CB = NBLK // NAG        # blocks per AllGather chunk
NSWQ = 4                # SWDGE queues for gather overlap

# colconst column layout (f32, each value replicated on all 128 partitions)
CC_GIN, CC_BIN = 0, 128
CC_G1, CC_B1, CC_BIAS1 = 256, 768, 1280
CC_G2, CC_B2, CC_BIAS2 = 1792, 1920, 2048
CC_BO = 2176
NCC = 2208


def _tid(n):
    """table row id of padded-node id n: single rank-major AllGather shard
    layout (core c's NPB local rows at [c*NPB, (c+1)*NPB))."""
    blk = n // P
    c = blk % NCORES
    l = blk // NCORES
    return c * NPB + l * P + n % P


def prepare_inputs(x, edge_index):
    """Host graph preprocessing -> per-core arrays + degree schedule."""
    x = np.asarray(x, dtype=np.float32)
    ei = np.asarray(edge_index)
    src = np.concatenate([ei[0], np.arange(N, dtype=ei.dtype)]).astype(np.int64)
    dst = np.concatenate([ei[1], np.arange(N, dtype=ei.dtype)]).astype(np.int64)

    deg = np.bincount(dst, minlength=N)
    order = np.argsort(deg, kind="stable")        # orig node ids, deg ascending
    newid = np.empty(N, dtype=np.int64)           # orig -> padded id
    newid[order] = np.arange(N) + (NPAD - N)      # pads occupy ids [0, 480)

    degp = np.zeros(NPAD, dtype=np.int64)
    degp[newid] = deg
    gmax = degp.reshape(NBLK_G, P).max(axis=1)
    ghat = gmax.reshape(NBLK, NCORES).max(axis=1)         # per local block idx
    S = int(P * ghat.sum())                                # slots per core

    # CSR of in-edges keyed by new dst id
    nd = newid[dst]
    csr_order = np.argsort(nd, kind="stable")
    nsrc_sorted = newid[src[csr_order]]
    indptr = np.zeros(NPAD + 1, dtype=np.int64)
    np.cumsum(np.bincount(nd, minlength=NPAD), out=indptr[1:])

    tid_of = _tid(np.arange(NPAD))

    goff = np.zeros(NBLK, dtype=np.int64)                  # k-slot offsets
    goff[1:] = np.cumsum(ghat)[:-1]

    idxw = np.zeros((NCORES, P, S // 16), dtype=np.int16)
    x_own = np.zeros((NCORES, NPB, D_IN), dtype=np.float32)
    alsfix = np.zeros((NCORES, NPB, 8), dtype=np.float32)

    inv_new = np.full(NPAD, -1, dtype=np.int64)
    inv_new[newid] = np.arange(N)

    for c in range(NCORES):
        gblk = np.arange(NBLK) * NCORES + c                # global block ids
        nid = (gblk[:, None] * P + np.arange(P)).reshape(-1)   # [NPB] padded id
        ov = inv_new[nid]                                  # orig node or -1
        real = ov >= 0
        x_own[c][real] = x[ov[real]]
        alsfix[c][~real, :] = NEG

        idx_flat = np.zeros(S, dtype=np.int16)             # dummy -> row 0
        for l in range(NBLK):
            d0 = nid[l * P:(l + 1) * P]                    # padded ids of block
            base = goff[l] * P
            for p in range(P):
                d = d0[p]
                s0, s1 = indptr[d], indptr[d + 1]
                ks = np.arange(s1 - s0)
                idx_flat[base + ks * P + p] = tid_of[nsrc_sorted[s0:s1]]
        idxw[c] = np.tile(idx_flat.reshape(S // 16, 16).T, (NCORES, 1))

    return {
        "ghat": [int(g) for g in ghat],
        "S": S,
        "idxw": idxw,
        "x_own": x_own,
        "alsfix": alsfix,
        "newid": newid,
    }


def prepare_weights(W1, att1_s, att1_d, bias1, g1, b1, g_in, b_in,
                    W2, att2_s, att2_d, bias2, g2, b2, Wo, bo):
    W1 = np.asarray(W1, np.float32)
    W2 = np.asarray(W2, np.float32)
    w1ext = np.zeros((D_IN, 520), dtype=BF16)
    w1ext[:, :512] = W1
    W1h = W1.reshape(D_IN, H1, HID)
    w1ext[:, 512:516] = np.einsum("khc,hc->kh", W1h, np.asarray(att1_s, np.float32))
    w1ext[:, 516:520] = np.einsum("khc,hc->kh", W1h, np.asarray(att1_d, np.float32))

    w2e = np.zeros((4 * HID, 130), dtype=np.float32)
    w2e[:, :128] = W2
    w2e[:, 128] = W2 @ np.asarray(att2_s, np.float32)[0]
    w2e[:, 129] = W2 @ np.asarray(att2_d, np.float32)[0]
    # pack [512, 130] -> [128, 4, 130] (partition p holds rows p, 128+p, ...)
    w2ext = np.ascontiguousarray(
        w2e.reshape(4, P, 130).transpose(1, 0, 2)).astype(BF16)

    woext = np.asarray(Wo, np.float32).astype(BF16)

    cc = np.zeros(NCC, dtype=np.float32)
    cc[CC_GIN:CC_GIN + 128] = g_in
    cc[CC_BIN:CC_BIN + 128] = b_in
    cc[CC_G1:CC_G1 + 512] = g1
    cc[CC_B1:CC_B1 + 512] = b1
    cc[CC_BIAS1:CC_BIAS1 + 512] = bias1
    cc[CC_G2:CC_G2 + 128] = g2
    cc[CC_B2:CC_B2 + 128] = b2
    cc[CC_BIAS2:CC_BIAS2 + 128] = bias2
    cc[CC_BO:CC_BO + 32] = bo
    colconst = np.tile(cc[None, :], (P, 1))

    return {"w1ext": w1ext, "w2ext": w2ext.reshape(P, 4 * 130),
            "woext": woext, "colconst": colconst}


def _bap(ap, dims):
    """AP with explicit free-dim [step, count] pairs (partition dim kept)."""
    return bass.AP(ap.tensor, ap.offset, [ap.ap[0]] + [list(d) for d in dims])


def build_program(ghat, triv=(), num_devices=NCORES, shared_ag=True):
    """triv: subset of {'gb0','g1b1','g2b2','bias1','bias2','bo'} marking
    affine/bias params that are identity/zero for this input set (detected on
    host); the corresponding device ops are skipped."""
    goff = np.zeros(NBLK, dtype=np.int64)
    goff[1:] = np.cumsum(ghat)[:-1]
    triv = set(triv)
    # process heavy blocks first so the last chunk (pipeline drain before the
    # next AllGather) is the cheapest
    order = sorted(range(NBLK), key=lambda l: -ghat[l])

    nc = bacc.Bacc("TRN2", target_bir_lowering=False, debug=False,
                   num_devices=num_devices, num_swdge_queues=NSWQ)

    x_own = nc.dram_tensor("x_own", [NPB, D_IN], F32, kind="ExternalInput")
    idxw = nc.dram_tensor("idxw", [P, int(P * sum(ghat)) // 16], I16,
                          kind="ExternalInput")
    alsfix = nc.dram_tensor("alsfix", [NPB, 8], F32, kind="ExternalInput")
    w1ext = nc.dram_tensor("w1ext", [D_IN, 520], BF, kind="ExternalInput")
    w2ext = nc.dram_tensor("w2ext", [P, 4 * 130], BF, kind="ExternalInput")
    woext = nc.dram_tensor("woext", [P, D_OUT], BF, kind="ExternalInput")
    colconst = nc.dram_tensor("colconst", [P, NCC], F32, kind="ExternalInput")
    out = nc.dram_tensor("out", [NPB, D_OUT], F32, kind="ExternalOutput")

    rg = [list(range(num_devices))]
    qrr = [0]
    ag_space = "Shared" if shared_ag else "Local"
    S16 = int(P * sum(ghat)) // 16

    with tile.TileContext(nc) as tc:
        with (
            tc.tile_pool(name="cst", bufs=1) as cst,
            tc.tile_pool(name="wp", bufs=3) as wp,
            tc.tile_pool(name="gp1", bufs=5) as gp1,
            tc.tile_pool(name="wtp", bufs=3) as wtp,
            tc.tile_pool(name="gp2", bufs=8) as gp2,
            tc.tile_pool(name="wt2p", bufs=4) as wt2p,
            tc.tile_pool(name="hb", bufs=2) as hb,
            tc.tile_pool(name="vsp", bufs=2) as vsp,
            tc.tile_pool(name="ps", bufs=2, space="PSUM") as ps,
            tc.tile_pool(name="pss", bufs=3, space="PSUM") as pss,
            tc.tile_pool(name="dram", bufs=1, space="DRAM") as dram,
        ):
            # ---- constants ----
            ident = cst.tile([P, P], BF)
            make_identity(nc, ident[:])
            w1s = cst.tile([P, 520], BF)
            nc.sync.dma_start(w1s[:], w1ext[:])
            w2s = cst.tile([P, 4, 130], BF)
            nc.sync.dma_start(w2s[:], w2ext[:])
            wos = cst.tile([P, D_OUT], BF)
            nc.sync.dma_start(wos[:], woext[:])
            cc = cst.tile([P, NCC], F32)
            nc.sync.dma_start(cc[:], colconst[:])
            idx_sb = cst.tile([P, S16], I16)
            nc.sync.dma_start(idx_sb[:], idxw[:])
            afix = cst.tile([P, NBLK, 8], F32)
            nc.sync.dma_start(
                afix[:], bass.AP(alsfix.ap().tensor, 0,
                                 [[8, P], [8 * P, NBLK], [1, 8]]))
            eps_t = cst.tile([P, 1], F32)
            nc.vector.memset(eps_t[:], EPS)
            ald1 = cst.tile([P, NBLK, H1], F32)
            ald2 = cst.tile([P, NBLK, 1], F32)
            h2b_all = cst.tile([P, NBLK, HID], BF)
            zc_all = cst.tile([P, NBLK, D_OUT], F32)
            sden_all = cst.tile([P, NBLK], F32)
            sqdump = cst.tile([P, 512], BF)     # discarded Square outputs
            xall = cst.tile([P, NBLK, D_IN], F32)
            nc.sync.dma_start(
                xall[:], bass.AP(x_own.ap().tensor, 0,
                                 [[D_IN, P], [P * D_IN, NBLK], [1, D_IN]]))

            ag1_in = dram.tile([NPB, T1COLS], BF)
            ag1_out = dram.tile([NPAD, T1COLS], BF, addr_space=ag_space)
            ag2_in = dram.tile([NPB, T2COLS], BF)
            ag2_out = dram.tile([NPAD, T2COLS], BF, addr_space=ag_space)

            def transpose_to(dst_bf, src_bf):
                pst = pss.tile([P, P], BF, tag="tp")
                nc.tensor.transpose(out=pst[:], in_=src_bf, identity=ident[:])
                nc.vector.tensor_copy(out=dst_bf, in_=pst[:])

            def rstd_of(vs, scale, tag):
                """rstd[:,i] = (vs[:,i]*scale + eps)^(-1/2) via exp/ln."""
                lnv = wp.tile([P, CB], F32, tag=f"lnv{tag}")
                nc.scalar.activation(lnv[:], vs[:], AF.Ln, bias=eps_t[:],
                                     scale=scale)
                rst = wp.tile([P, CB], F32, tag=f"rst{tag}")
                nc.scalar.activation(rst[:], lnv[:], AF.Exp, scale=-0.5)
                return rst

            # ================= phase 0: LN0 + W1, build L1 table =============
            # square+sqrt live in one ACT table -> per-block rstd (sqrt then
            # DVE reciprocal), no chunk barrier, fully streamed.
            for t in range(NBLK):
                mu = wp.tile([P, 1], F32, tag="mu0")
                nc.vector.tensor_reduce(out=mu[:], in_=xall[:, t, :],
                                        axis=mybir.AxisListType.X, op=OP.add)
                nc.scalar.mul(mu[:], mu[:], 1.0 / D_IN)
                xc = wp.tile([P, D_IN], F32, tag="xc0")
                nc.vector.tensor_scalar_sub(out=xc[:], in0=xall[:, t, :],
                                            scalar1=mu[:])
                ss = wp.tile([P, 1], F32, tag="ss0")
                nc.scalar.activation(sqdump[:, 0:D_IN], xc[:], AF.Square,
                                     accum_out=ss[:])
                sd = wp.tile([P, 1], F32, tag="sd0")
                nc.scalar.activation(sd[:], ss[:], AF.Sqrt, bias=eps_t[:],
                                     scale=1.0 / D_IN)
                rst = wp.tile([P, 1], F32, tag="rst0")
                nc.vector.reciprocal(rst[:], sd[:])
                if "gb0" in triv:
                    xnb = wp.tile([P, D_IN], BF, tag="xnb")
                    nc.vector.tensor_scalar_mul(out=xnb[:], in0=xc[:],
                                                scalar1=rst[:])
                else:
                    xn = wp.tile([P, D_IN], F32, tag="xn0")
                    nc.vector.tensor_scalar_mul(out=xn[:], in0=xc[:],
                                                scalar1=rst[:])
                    xg = wp.tile([P, D_IN], F32, tag="xg0")
                    nc.vector.tensor_mul(out=xg[:], in0=xn[:],
                                         in1=cc[:, CC_GIN:CC_GIN + D_IN])
                    xnb = wp.tile([P, D_IN], BF, tag="xnb")
                    nc.vector.tensor_tensor(out=xnb[:], in0=xg[:],
                                            in1=cc[:, CC_BIN:CC_BIN + D_IN],
                                            op=OP.add)
                xT = wp.tile([P, P], BF, tag="xT")
                transpose_to(xT[:], xnb[:])
                ps1 = ps.tile([P, 512], F32, tag="big")
                nc.tensor.matmul(ps1[:], lhsT=xT[:], rhs=w1s[:, 0:512],
                                 start=True, stop=True)
                ps2_t = pss.tile([P, 130], F32, tag="mm2")
                ps2 = ps2_t[:, 0:8]
                nc.tensor.matmul(ps2[:], lhsT=xT[:], rhs=w1s[:, 512:520],
                                 start=True, stop=True)
                tt = wp.tile([P, T1USED], BF, tag="tt")
                nc.vector.tensor_copy(out=tt[:, 0:512], in_=ps1[:])
                nc.vector.tensor_tensor(
                    out=tt[:, 512:520].bitcast(F32), in0=ps2[:, 0:4],
                    in1=afix[:, t, 0:4], op=OP.add)
                nc.vector.tensor_copy(out=ald1[:, t, :], in_=ps2[:, 4:8])
                nc.sync.dma_start(
                    ag1_in[t * P:(t + 1) * P, 0:T1USED], tt[:])
            nc.gpsimd.collective_compute(
                "AllGather", OP.bypass, replica_groups=rg,
                ins=[ag1_in[:].opt()], outs=[ag1_out[:].opt()])

            # ================= phase 2: GAT layer 1 ==========================
            for j in range(NAG):
                vs = vsp.tile([P, CB], F32, tag="vs1")
                h5 = hb.tile([P, CB, 512], F32, tag="h5")
                for i in range(CB):
                    l = order[j * CB + i]
                    g = ghat[l]
                    psA = ps.tile([P, 512], F32, tag="big")
                    den = wp.tile([P, H1], F32, tag="den1")
                    k0 = 0
                    while k0 < g:
                        kn = min(KC, g - k0)
                        gt = gp1.tile([P, KC, T1COLS], BF, tag="g1")
                        nc.gpsimd.dma_gather(
                            gt[:, 0:kn, :], ag1_out[:],
                            idx_sb[:, 8 * (int(goff[l]) + k0):
                                   8 * (int(goff[l]) + k0 + kn)],
                            P * kn, P * kn, T1COLS, single_packet=False,
                            queue_num=qrr[0] % NSWQ)
                        qrr[0] += 1
                        als_v = gt[:, 0:kn, 512:520].bitcast(F32)
                        u = wp.tile([P, KC, H1], F32, tag="u1")
                        nc.vector.tensor_tensor(
                            out=u[:, 0:kn, :], in0=als_v,
                            in1=_bap(ald1[:, l, :], [(0, kn), (1, H1)]),
                            op=OP.add)
                        lk = wp.tile([P, KC, H1], F32, tag="lk1")
                        nc.scalar.activation(lk[:, 0:kn, :], u[:, 0:kn, :],
                                             AF.Prelu, alpha=0.2)
                        exb = wp.tile([P, KC, H1], BF, tag="exb1")
                        nc.scalar.activation(exb[:, 0:kn, :], lk[:, 0:kn, :],
                                             AF.Exp)
                        dt_ = wp.tile([P, H1], F32, tag="dt1")
                        red = dt_ if k0 else den
                        nc.vector.tensor_reduce(
                            out=red[:], in_=_bap(exb[:], [(1, H1), (H1, kn)]),
                            axis=mybir.AxisListType.X, op=OP.add)
                        if k0:
                            dn2 = wp.tile([P, H1], F32, tag="dn1b")
                            nc.vector.tensor_add(dn2[:], den[:], dt_[:])
                            den = dn2
                        wt = wtp.tile([P, KC, 512], BF, tag="w1")
                        nc.vector.tensor_tensor(
                            out=wt[:, 0:kn, :],
                            in0=_bap(gt[:], [(T1COLS, kn), (HID, H1),
                                             (1, HID)]),
                            in1=_bap(exb[:], [(H1, kn), (1, H1), (0, HID)]),
                            op=OP.mult)
                        for k in range(kn):
                            nc.tensor.matmul(psA[:], lhsT=ident[:],
                                             rhs=wt[:, k, :],
                                             start=(k0 + k == 0),
                                             stop=(k0 + k == g - 1))
                        k0 += kn
                    dne = wp.tile([P, H1], F32, tag="dne1")
                    nc.vector.tensor_scalar_add(out=dne[:], in0=den[:],
                                                scalar1=1e-30)
                    denr = wp.tile([P, H1], F32, tag="dr1")
                    nc.vector.reciprocal(denr[:], dne[:])
                    for h in range(H1):
                        nc.scalar.activation(
                            h5[:, i, h * HID:(h + 1) * HID],
                            psA[:, h * HID:(h + 1) * HID], AF.Copy,
                            scale=denr[:, h:h + 1])
                # chunk-batched LN: one wide op per step instead of 5 narrow
                if "bias1" not in triv:
                    nc.vector.tensor_tensor(
                        out=h5[:], in0=h5[:],
                        in1=_bap(cc[:, CC_BIAS1:CC_BIAS1 + 512],
                                 [(0, CB), (1, 512)]), op=OP.add)
                mu5 = wp.tile([P, CB], F32, tag="mu1")
                nc.vector.tensor_reduce(
                    out=mu5[:], in_=_bap(h5[:], [(512, CB), (1, 512)]),
                    axis=mybir.AxisListType.X, op=OP.add)
                nc.scalar.mul(mu5[:], mu5[:], 1.0 / 512)
                nc.vector.tensor_tensor(
                    out=h5[:], in0=h5[:],
                    in1=_bap(mu5[:], [(1, CB), (0, 512)]), op=OP.subtract)
                for i in range(CB):
                    nc.scalar.activation(sqdump[:], h5[:, i, :], AF.Square,
                                         accum_out=vs[:, i:i + 1])
                rst = rstd_of(vs, 1.0 / 512, "1")
                nc.vector.tensor_tensor(
                    out=h5[:], in0=h5[:],
                    in1=_bap(rst[:], [(1, CB), (0, 512)]), op=OP.mult)
                if "g1b1" not in triv:
                    nc.vector.tensor_tensor(
                        out=h5[:], in0=h5[:],
                        in1=_bap(cc[:, CC_G1:CC_G1 + 512],
                                 [(0, CB), (1, 512)]), op=OP.mult)
                    nc.vector.tensor_tensor(
                        out=h5[:], in0=h5[:],
                        in1=_bap(cc[:, CC_B1:CC_B1 + 512],
                                 [(0, CB), (1, 512)]), op=OP.add)
                h1b5 = hb.tile([P, CB, 512], BF, tag="h1b5")
                nc.scalar.activation(h1b5[:], h5[:], AF.Gelu)
                for i in range(CB):
                    l = order[j * CB + i]
                    ps3 = pss.tile([P, 130], F32, tag="mm2")
                    for cch in range(4):
                        hT = wp.tile([P, P], BF, tag="hT")
                        transpose_to(hT[:], h1b5[:, i, cch * P:(cch + 1) * P])
                        nc.tensor.matmul(ps3[:], lhsT=hT[:], rhs=w2s[:, cch, :],
                                         start=(cch == 0), stop=(cch == 3))
                    t2 = wp.tile([P, T2USED], BF, tag="t2")
                    nc.vector.tensor_copy(out=t2[:, 0:128], in_=ps3[:, 0:128])
                    nc.vector.tensor_tensor(
                        out=t2[:, 128:130].bitcast(F32), in0=ps3[:, 128:129],
                        in1=afix[:, l, 4:5], op=OP.add)
                    nc.vector.tensor_copy(out=ald2[:, l, :], in_=ps3[:, 129:130])
                    nc.sync.dma_start(
                        ag2_in[l * P:(l + 1) * P, 0:T2USED], t2[:])
            nc.gpsimd.collective_compute(
                "AllGather", OP.bypass, replica_groups=rg,
                ins=[ag2_in[:].opt()], outs=[ag2_out[:].opt()])

            # ================= phase 4: GAT layer 2 ==========================
            # single head: leaky(als+ald) folds into ACT Prelu (per-partition
            # ald2 bias), exp's accum_out yields the softmax denominator
            for j in range(NAG):
                vs = vsp.tile([P, CB], F32, tag="vs2")
                h5b = hb.tile([P, CB, 128], F32, tag="h5b")
                for i in range(CB):
                    l = order[j * CB + i]
                    g = ghat[l]
                    psB_t = ps.tile([P, 512], F32, tag="big")
                    psB = psB_t[:, 0:128]
                    den = None
                    k0 = 0
                    while k0 < g:
                        kn = min(KC, g - k0)
                        gt = gp2.tile([P, KC, T2COLS], BF, tag="g2")
                        nc.gpsimd.dma_gather(
                            gt[:, 0:kn, :], ag2_out[:],
                            idx_sb[:, 8 * (int(goff[l]) + k0):
                                   8 * (int(goff[l]) + k0 + kn)],
                            P * kn, P * kn, T2COLS, single_packet=False,
                            queue_num=qrr[0] % NSWQ)
                        qrr[0] += 1
                        als_v = gt[:, 0:kn, 128:130].bitcast(F32)
                        lk = wp.tile([P, KC, 1], F32, tag="lk2")
                        nc.scalar.activation(lk[:, 0:kn, :], als_v,
                                             AF.Prelu, bias=ald2[:, l, :],
                                             alpha=0.2)
                        exb = wp.tile([P, KC, 1], BF, tag="exb2")
                        dt_ = wp.tile([P, 1], F32, tag="dt2")
                        nc.scalar.activation(exb[:, 0:kn, :], lk[:, 0:kn, :],
                                             AF.Exp, accum_out=dt_[:])
                        if k0:
                            dn2 = wp.tile([P, 1], F32, tag="dn2b")
                            nc.vector.tensor_add(dn2[:], den[:], dt_[:])
                            den = dn2
                        else:
                            den = dt_
                        wt = wt2p.tile([P, KC, 128], BF, tag="w2")
                        nc.vector.tensor_tensor(
                            out=wt[:, 0:kn, :],
                            in0=_bap(gt[:], [(T2COLS, kn), (1, 128)]),
                            in1=_bap(exb[:], [(1, kn), (0, 128)]),
                            op=OP.mult)
                        for k in range(kn):
                            nc.tensor.matmul(psB[:], lhsT=ident[:],
                                             rhs=wt[:, k, :],
                                             start=(k0 + k == 0),
                                             stop=(k0 + k == g - 1))
                        k0 += kn
                    dne = wp.tile([P, 1], F32, tag="dne2")
                    nc.vector.tensor_scalar_add(out=dne[:], in0=den[:],
                                                scalar1=1e-30)
                    denr = wp.tile([P, 1], F32, tag="dr2")
                    nc.vector.reciprocal(denr[:], dne[:])
                    nc.vector.tensor_scalar_mul(out=h5b[:, i, :], in0=psB[:],
                                                scalar1=denr[:])
                if "bias2" not in triv:
                    nc.vector.tensor_tensor(
                        out=h5b[:], in0=h5b[:],
                        in1=_bap(cc[:, CC_BIAS2:CC_BIAS2 + 128],
                                 [(0, CB), (1, 128)]), op=OP.add)
                mu5 = wp.tile([P, CB], F32, tag="mu2")
                nc.vector.tensor_reduce(
                    out=mu5[:], in_=_bap(h5b[:], [(128, CB), (1, 128)]),
                    axis=mybir.AxisListType.X, op=OP.add)
                nc.scalar.mul(mu5[:], mu5[:], 1.0 / 128)
                nc.vector.tensor_tensor(
                    out=h5b[:], in0=h5b[:],
                    in1=_bap(mu5[:], [(1, CB), (0, 128)]), op=OP.subtract)
                for i in range(CB):
                    nc.scalar.activation(sqdump[:, 0:128], h5b[:, i, :],
                                         AF.Square, accum_out=vs[:, i:i + 1])
                rst = rstd_of(vs, 1.0 / 128, "2")
                nc.vector.tensor_tensor(
                    out=h5b[:], in0=h5b[:],
                    in1=_bap(rst[:], [(1, CB), (0, 128)]), op=OP.mult)
                if "g2b2" not in triv:
                    nc.vector.tensor_tensor(
                        out=h5b[:], in0=h5b[:],
                        in1=_bap(cc[:, CC_G2:CC_G2 + 128],
                                 [(0, CB), (1, 128)]), op=OP.mult)
                    nc.vector.tensor_tensor(
                        out=h5b[:], in0=h5b[:],
                        in1=_bap(cc[:, CC_B2:CC_B2 + 128],
                                 [(0, CB), (1, 128)]), op=OP.add)
                nc.scalar.activation(h2b_all[:, j * CB:(j + 1) * CB, :],
                                     h5b[:], AF.Gelu)
                for i in range(CB):
                    pos = j * CB + i
                    l = order[pos]
                    hoT = wp.tile([P, P], BF, tag="hoT")
                    transpose_to(hoT[:], h2b_all[:, pos, :])
                    pso_t = pss.tile([P, 130], F32, tag="mm2")
                    pso = pso_t[:, 0:D_OUT]
                    nc.tensor.matmul(pso[:], lhsT=hoT[:], rhs=wos[:],
                                     start=True, stop=True)
                    if "bo" in triv:
                        z = pso
                    else:
                        z = wp.tile([P, D_OUT], F32, tag="z")
                        nc.vector.tensor_tensor(out=z[:], in0=pso[:],
                                                in1=cc[:, CC_BO:CC_BO + 32],
                                                op=OP.add)
                    m = wp.tile([P, 1], F32, tag="zm")
                    nc.vector.tensor_reduce(out=m[:], in_=z[:],
                                            axis=mybir.AxisListType.X,
                                            op=OP.max)
                    nc.vector.tensor_scalar_sub(out=zc_all[:, l, :], in0=z[:],
                                                scalar1=m[:])
                    ez = wp.tile([P, D_OUT], F32, tag="ez")
                    nc.scalar.activation(ez[:], zc_all[:, l, :], AF.Exp,
                                         accum_out=sden_all[:, l:l + 1])

            # ================= log_softmax tail ==============================
            for l in order:
                lnd = wp.tile([P, 1], F32, tag="lnd")  # noqa: order-iter
                nc.scalar.activation(lnd[:], sden_all[:, l:l + 1], AF.Ln)
                res = wp.tile([P, D_OUT], F32, tag="res")
                nc.vector.tensor_scalar_sub(out=res[:], in0=zc_all[:, l, :],
                                            scalar1=lnd[:])
                nc.sync.dma_start(out[l * P:(l + 1) * P, :], res[:])

    nc.compile()
    return nc


_CACHE = {}
_LAST_RUN = {}


def kernel(x, edge_index, g_in, b_in, W1, att1_s, att1_d, bias1, g1, b1,
           W2, att2_s, att2_d, bias2, g2, b2, Wo, bo):
    prep = prepare_inputs(x, edge_index)
    wts = prepare_weights(W1, att1_s, att1_d, bias1, g1, b1, g_in, b_in,
                          W2, att2_s, att2_d, bias2, g2, b2, Wo, bo)

    triv = []
    if np.allclose(g_in, 1) and np.allclose(b_in, 0):
        triv.append("gb0")
    if np.allclose(g1, 1) and np.allclose(b1, 0):
        triv.append("g1b1")
    if np.allclose(g2, 1) and np.allclose(b2, 0):
        triv.append("g2b2")
    if np.allclose(bias1, 0):
        triv.append("bias1")
    if np.allclose(bias2, 0):
        triv.append("bias2")
    if np.allclose(bo, 0):
        triv.append("bo")
    triv = tuple(sorted(triv))

    key = (tuple(prep["ghat"]), triv)
    if key not in _CACHE:
        _CACHE[key] = build_program(prep["ghat"], triv=triv)
    nc = _CACHE[key]

    in_maps = []
    for c in range(NCORES):
        in_maps.append({
            "x_own": prep["x_own"][c],
            "idxw": prep["idxw"][c],
            "alsfix": prep["alsfix"][c],
            "w1ext": wts["w1ext"],
            "w2ext": wts["w2ext"].astype(BF16),
            "woext": wts["woext"],
            "colconst": wts["colconst"],
        })

    _LAST_RUN.update(nc=nc, in_maps=in_maps, prep=prep)
    res = bass_utils.run_bass_kernel_spmd(nc, in_maps,
                                          core_ids=list(range(NCORES)))
    outs = [res.results[c]["out"] for c in range(NCORES)]

    newid = prep["newid"]
    blk = newid // P
    core = blk % NCORES
    row = (blk // NCORES) * P + newid % P
    full = np.empty((N, D_OUT), dtype=np.float32)
    for c in range(NCORES):
        sel = core == c
        full[sel] = outs[c][row[sel]]
    return full


# revision 24
# speedup vs baseline: 1.0577x; 1.0577x over previous
"""Distributed 2-layer GAT kernel for 8 Trainium2 NeuronCores.

Strategy (host graph preprocessing + device SPMD kernel):
  * Nodes are relabeled by in-degree (ascending) and padded to 20480 ids.
    Blocks of 128 consecutive ids then have near-uniform in-degree, and the
    160 blocks are dealt round-robin to the 8 cores (core = block % 8), so
    every core sees the same per-block degree schedule ghat[l] (compile-time
    constant -> identical SPMD program; all per-core variation is in data).
  * Edge slots are dst-major: slot (block l, k, partition p) holds the k-th
    in-edge of dst p in block l.  A dma_gather pulls the k-th edge row of all
    128 dsts into one [128, cols] SBUF tile (row lands on its dst partition),
    so attention softmax needs no index math on device and the sum over
    in-edges is PSUM accumulation with a constant identity stationary matrix.
  * Each layer's per-node table row = [features bf16 | a_src f32] is built by
    the owning core and AllGather'd (Shared-output HBM scratchpad) so gathers
    are core-local.  AllGather chunks are interleaved with the producer
    blocks so the collective overlaps compute.
  * Pad edge slots point to table row 0, whose a_src is forced to -1e9 on
    host data (alsfix), making exp(leaky_relu(...)) == 0 exactly.
  * Per NAG-chunk 3-pass structure keeps each ACT-table function grouped
    (attention exp + LN stats | batched rstd via exp(-0.5*ln(var+eps)) |
    gelu + next-layer matmul) so activation-table reloads happen per chunk,
    not per block.
"""
import sys

sys.path.insert(0, "/opt/trn_rl_repo")

import numpy as np
import ml_dtypes

from concourse import bass, bacc, tile, mybir
from concourse import bass_utils
from concourse.masks import make_identity

BF16 = ml_dtypes.bfloat16
F32 = mybir.dt.float32
BF = mybir.dt.bfloat16
I16 = mybir.dt.int16
AF = mybir.ActivationFunctionType
OP = mybir.AluOpType

# problem constants
N, E = 20000, 320000
D_IN, HID, D_OUT = 128, 128, 32
H1, H2 = 4, 1
EPS = 1e-5

NCORES = 8
P = 128
NPAD = 20480            # padded node count: 160 blocks of 128
NBLK_G = NPAD // P      # 160 global blocks
NPB = NPAD // NCORES    # 2560 nodes per core
NBLK = NPB // P         # 20 blocks per core
NEG = -1e9

T1COLS = 640            # L1 table row (bf16): 512 feats | 4 f32 a_src | pad
T1USED = 520
T2COLS = 256            # L2 table row (bf16): 128 feats | 1 f32 a_src | pad
T2USED = 130
KC = 8                  # max in-edge slots per gather call
NAG = 4                 # AllGather chunks (interleaved with producers)
CB = NBLK // NAG        # blocks per AllGather chunk
NSWQ = 4                # SWDGE queues for gather overlap

# colconst column layout (f32, each value replicated on all 128 partitions)
CC_GIN, CC_BIN = 0, 128
CC_G1, CC_B1, CC_BIAS1 = 256, 768, 1280
CC_G2, CC_B2, CC_BIAS2 = 1792, 1920, 2048
CC_BO = 2176
NCC = 2208


def _tid(n):
    """table row id of padded-node id n: single rank-major AllGather shard
    layout (core c's NPB local rows at [c*NPB, (c+1)*NPB))."""
    blk = n // P
    c = blk % NCORES
    l = blk // NCORES
    return c * NPB + l * P + n % P


def prepare_inputs(x, edge_index):
    """Host graph preprocessing -> per-core arrays + degree schedule."""
    x = np.asarray(x, dtype=np.float32)
    ei = np.asarray(edge_index)
    src = np.concatenate([ei[0], np.arange(N, dtype=ei.dtype)]).astype(np.int64)
    dst = np.concatenate([ei[1], np.arange(N, dtype=ei.dtype)]).astype(np.int64)

    deg = np.bincount(dst, minlength=N)
    order = np.argsort(deg, kind="stable")        # orig node ids, deg ascending
    newid = np.empty(N, dtype=np.int64)           # orig -> padded id
    newid[order] = np.arange(N) + (NPAD - N)      # pads occupy ids [0, 480)

    degp = np.zeros(NPAD, dtype=np.int64)
    degp[newid] = deg
    gmax = degp.reshape(NBLK_G, P).max(axis=1)
    ghat = gmax.reshape(NBLK, NCORES).max(axis=1)         # per local block idx
    S = int(P * ghat.sum())                                # slots per core

    # CSR of in-edges keyed by new dst id
    nd = newid[dst]
    csr_order = np.argsort(nd, kind="stable")
    nsrc_sorted = newid[src[csr_order]]
    indptr = np.zeros(NPAD + 1, dtype=np.int64)
    np.cumsum(np.bincount(nd, minlength=NPAD), out=indptr[1:])

    tid_of = _tid(np.arange(NPAD))

    goff = np.zeros(NBLK, dtype=np.int64)                  # k-slot offsets
    goff[1:] = np.cumsum(ghat)[:-1]

    idxw = np.zeros((NCORES, P, S // 16), dtype=np.int16)
    x_own = np.zeros((NCORES, NPB, D_IN), dtype=np.float32)
    alsfix = np.zeros((NCORES, NPB, 8), dtype=np.float32)

    inv_new = np.full(NPAD, -1, dtype=np.int64)
    inv_new[newid] = np.arange(N)

    for c in range(NCORES):
        gblk = np.arange(NBLK) * NCORES + c                # global block ids
        nid = (gblk[:, None] * P + np.arange(P)).reshape(-1)   # [NPB] padded id
        ov = inv_new[nid]                                  # orig node or -1
        real = ov >= 0
        x_own[c][real] = x[ov[real]]
        alsfix[c][~real, :] = NEG

        idx_flat = np.zeros(S, dtype=np.int16)             # dummy -> row 0
        for l in range(NBLK):
            d0 = nid[l * P:(l + 1) * P]                    # padded ids of block
            base = goff[l] * P
            for p in range(P):
                d = d0[p]
                s0, s1 = indptr[d], indptr[d + 1]
                ks = np.arange(s1 - s0)
                idx_flat[base + ks * P + p] = tid_of[nsrc_sorted[s0:s1]]
        idxw[c] = np.tile(idx_flat.reshape(S // 16, 16).T, (NCORES, 1))

    return {
        "ghat": [int(g) for g in ghat],
        "S": S,
        "idxw": idxw,
        "x_own": x_own,
        "alsfix": alsfix,
        "newid": newid,
    }


def prepare_weights(W1, att1_s, att1_d, bias1, g1, b1, g_in, b_in,
                    W2, att2_s, att2_d, bias2, g2, b2, Wo, bo):
    W1 = np.asarray(W1, np.float32)
    W2 = np.asarray(W2, np.float32)
    w1ext = np.zeros((D_IN, 520), dtype=BF16)
    w1ext[:, :512] = W1
    W1h = W1.reshape(D_IN, H1, HID)
    w1ext[:, 512:516] = np.einsum("khc,hc->kh", W1h, np.asarray(att1_s, np.float32))
    w1ext[:, 516:520] = np.einsum("khc,hc->kh", W1h, np.asarray(att1_d, np.float32))

    w2e = np.zeros((4 * HID, 130), dtype=np.float32)
    w2e[:, :128] = W2
    w2e[:, 128] = W2 @ np.asarray(att2_s, np.float32)[0]
    w2e[:, 129] = W2 @ np.asarray(att2_d, np.float32)[0]
    # pack [512, 130] -> [128, 4, 130] (partition p holds rows p, 128+p, ...)
    w2ext = np.ascontiguousarray(
        w2e.reshape(4, P, 130).transpose(1, 0, 2)).astype(BF16)

    woext = np.asarray(Wo, np.float32).astype(BF16)

    cc = np.zeros(NCC, dtype=np.float32)
    cc[CC_GIN:CC_GIN + 128] = g_in
    cc[CC_BIN:CC_BIN + 128] = b_in
    cc[CC_G1:CC_G1 + 512] = g1
    cc[CC_B1:CC_B1 + 512] = b1
    cc[CC_BIAS1:CC_BIAS1 + 512] = bias1
    cc[CC_G2:CC_G2 + 128] = g2
    cc[CC_B2:CC_B2 + 128] = b2
    cc[CC_BIAS2:CC_BIAS2 + 128] = bias2
    cc[CC_BO:CC_BO + 32] = bo
    colconst = np.tile(cc[None, :], (P, 1))

    return {"w1ext": w1ext, "w2ext": w2ext.reshape(P, 4 * 130),
            "woext": woext, "colconst": colconst}


def _bap(ap, dims):
    """AP with explicit free-dim [step, count] pairs (partition dim kept)."""
    return bass.AP(ap.tensor, ap.offset, [ap.ap[0]] + [list(d) for d in dims])


def build_program(ghat, triv=(), num_devices=NCORES, shared_ag=True):
    """triv: subset of {'gb0','g1b1','g2b2','bias1','bias2','bo'} marking
    affine/bias params that are identity/zero for this input set (detected on
    host); the corresponding device ops are skipped."""
    goff = np.zeros(NBLK, dtype=np.int64)
    goff[1:] = np.cumsum(ghat)[:-1]
    triv = set(triv)
    # process heavy blocks first so the last chunk (pipeline drain before the
    # next AllGather) is the cheapest
    order = sorted(range(NBLK), key=lambda l: -ghat[l])

    nc = bacc.Bacc("TRN2", target_bir_lowering=False, debug=False,
                   num_devices=num_devices, num_swdge_queues=NSWQ)

    x_own = nc.dram_tensor("x_own", [NPB, D_IN], F32, kind="ExternalInput")
    idxw = nc.dram_tensor("idxw", [P, int(P * sum(ghat)) // 16], I16,
                          kind="ExternalInput")
    alsfix = nc.dram_tensor("alsfix", [NPB, 8], F32, kind="ExternalInput")
    w1ext = nc.dram_tensor("w1ext", [D_IN, 520], BF, kind="ExternalInput")
    w2ext = nc.dram_tensor("w2ext", [P, 4 * 130], BF, kind="ExternalInput")
    woext = nc.dram_tensor("woext", [P, D_OUT], BF, kind="ExternalInput")
    colconst = nc.dram_tensor("colconst", [P, NCC], F32, kind="ExternalInput")
    out = nc.dram_tensor("out", [NPB, D_OUT], F32, kind="ExternalOutput")

    rg = [list(range(num_devices))]
    qrr = [0]
    ag_space = "Shared" if shared_ag else "Local"
    S16 = int(P * sum(ghat)) // 16

    with tile.TileContext(nc) as tc:
        with (
            tc.tile_pool(name="cst", bufs=1) as cst,
            tc.tile_pool(name="wp", bufs=3) as wp,
            tc.tile_pool(name="gp1", bufs=5) as gp1,
            tc.tile_pool(name="wtp", bufs=3) as wtp,
            tc.tile_pool(name="gp2", bufs=8) as gp2,
            tc.tile_pool(name="wt2p", bufs=4) as wt2p,
            tc.tile_pool(name="hb", bufs=2) as hb,
            tc.tile_pool(name="vsp", bufs=2) as vsp,
            tc.tile_pool(name="ps", bufs=2, space="PSUM") as ps,
            tc.tile_pool(name="pss", bufs=3, space="PSUM") as pss,
            tc.tile_pool(name="dram", bufs=1, space="DRAM") as dram,
        ):
            # ---- constants ----
            ident = cst.tile([P, P], BF)
            make_identity(nc, ident[:])
            w1s = cst.tile([P, 520], BF)
            nc.sync.dma_start(w1s[:], w1ext[:])
            w2s = cst.tile([P, 4, 130], BF)
            nc.sync.dma_start(w2s[:], w2ext[:])
            wos = cst.tile([P, D_OUT], BF)
            nc.sync.dma_start(wos[:], woext[:])
            cc = cst.tile([P, NCC], F32)
            nc.sync.dma_start(cc[:], colconst[:])
            idx_sb = cst.tile([P, S16], I16)
            nc.sync.dma_start(idx_sb[:], idxw[:])
            afix = cst.tile([P, NBLK, 8], F32)
            nc.sync.dma_start(
                afix[:], bass.AP(alsfix.ap().tensor, 0,
                                 [[8, P], [8 * P, NBLK], [1, 8]]))
            eps_t = cst.tile([P, 1], F32)
            nc.vector.memset(eps_t[:], EPS)
            ald1 = cst.tile([P, NBLK, H1], F32)
            ald2 = cst.tile([P, NBLK, 1], F32)
            h2b_all = cst.tile([P, NBLK, HID], BF)
            zc_all = cst.tile([P, NBLK, D_OUT], F32)
            sden_all = cst.tile([P, NBLK], F32)
            sqdump = cst.tile([P, 512], BF)     # discarded Square outputs
            xall = cst.tile([P, NBLK, D_IN], F32)
            nc.sync.dma_start(
                xall[:], bass.AP(x_own.ap().tensor, 0,
                                 [[D_IN, P], [P * D_IN, NBLK], [1, D_IN]]))

            ag1_in = dram.tile([NPB, T1COLS], BF)
            ag1_out = dram.tile([NPAD, T1COLS], BF, addr_space=ag_space)
            ag2_in = dram.tile([NPB, T2COLS], BF)
            ag2_out = dram.tile([NPAD, T2COLS], BF, addr_space=ag_space)

            def transpose_to(dst_bf, src_bf):
                pst = pss.tile([P, P], BF, tag="tp")
                nc.tensor.transpose(out=pst[:], in_=src_bf, identity=ident[:])
                nc.vector.tensor_copy(out=dst_bf, in_=pst[:])

            def rstd_of(vs, scale, tag):
                """rstd[:,i] = (vs[:,i]*scale + eps)^(-1/2) via exp/ln."""
                lnv = wp.tile([P, CB], F32, tag=f"lnv{tag}")
                nc.scalar.activation(lnv[:], vs[:], AF.Ln, bias=eps_t[:],
                                     scale=scale)
                rst = wp.tile([P, CB], F32, tag=f"rst{tag}")
                nc.scalar.activation(rst[:], lnv[:], AF.Exp, scale=-0.5)
                return rst

            # ================= phase 0: LN0 + W1, build L1 table =============
            # square+sqrt live in one ACT table -> per-block rstd (sqrt then
            # DVE reciprocal), no chunk barrier, fully streamed.
            for t in range(NBLK):
                mu = wp.tile([P, 1], F32, tag="mu0")
                nc.vector.tensor_reduce(out=mu[:], in_=xall[:, t, :],
                                        axis=mybir.AxisListType.X, op=OP.add)
                nc.scalar.mul(mu[:], mu[:], 1.0 / D_IN)
                xc = wp.tile([P, D_IN], F32, tag="xc0")
                nc.vector.tensor_scalar_sub(out=xc[:], in0=xall[:, t, :],
                                            scalar1=mu[:])
                ss = wp.tile([P, 1], F32, tag="ss0")
                nc.scalar.activation(sqdump[:, 0:D_IN], xc[:], AF.Square,
                                     accum_out=ss[:])
                sd = wp.tile([P, 1], F32, tag="sd0")
                nc.scalar.activation(sd[:], ss[:], AF.Sqrt, bias=eps_t[:],
                                     scale=1.0 / D_IN)
                rst = wp.tile([P, 1], F32, tag="rst0")
                nc.vector.reciprocal(rst[:], sd[:])
                if "gb0" in triv:
                    xnb = wp.tile([P, D_IN], BF, tag="xnb")
                    nc.vector.tensor_scalar_mul(out=xnb[:], in0=xc[:],
                                                scalar1=rst[:])
                else:
                    xn = wp.tile([P, D_IN], F32, tag="xn0")
                    nc.vector.tensor_scalar_mul(out=xn[:], in0=xc[:],
                                                scalar1=rst[:])
                    xg = wp.tile([P, D_IN], F32, tag="xg0")
                    nc.vector.tensor_mul(out=xg[:], in0=xn[:],
                                         in1=cc[:, CC_GIN:CC_GIN + D_IN])
                    xnb = wp.tile([P, D_IN], BF, tag="xnb")
                    nc.vector.tensor_tensor(out=xnb[:], in0=xg[:],
                                            in1=cc[:, CC_BIN:CC_BIN + D_IN],
                                            op=OP.add)
                xT = wp.tile([P, P], BF, tag="xT")
                transpose_to(xT[:], xnb[:])
                ps1 = ps.tile([P, 512], F32, tag="big")
                nc.tensor.matmul(ps1[:], lhsT=xT[:], rhs=w1s[:, 0:512],
                                 start=True, stop=True)
                ps2_t = pss.tile([P, 130], F32, tag="mm2")
                ps2 = ps2_t[:, 0:8]
                nc.tensor.matmul(ps2[:], lhsT=xT[:], rhs=w1s[:, 512:520],
                                 start=True, stop=True)
                tt = wp.tile([P, T1USED], BF, tag="tt")
                nc.vector.tensor_copy(out=tt[:, 0:512], in_=ps1[:])
                nc.vector.tensor_tensor(
                    out=tt[:, 512:520].bitcast(F32), in0=ps2[:, 0:4],
                    in1=afix[:, t, 0:4], op=OP.add)
                nc.vector.tensor_copy(out=ald1[:, t, :], in_=ps2[:, 4:8])
                nc.sync.dma_start(
                    ag1_in[t * P:(t + 1) * P, 0:T1USED], tt[:])
            nc.gpsimd.collective_compute(
                "AllGather", OP.bypass, replica_groups=rg,
                ins=[ag1_in[:].opt()], outs=[ag1_out[:].opt()])

            # ================= phase 2: GAT layer 1 ==========================
            for j in range(NAG):
                vs = vsp.tile([P, CB], F32, tag="vs1")
                h5 = hb.tile([P, CB, 512], F32, tag="h5")
                for i in range(CB):
                    l = order[j * CB + i]
                    g = ghat[l]
                    psA = ps.tile([P, 512], F32, tag="big")
                    den = wp.tile([P, H1], F32, tag="den1")
                    k0 = 0
                    while k0 < g:
                        kn = min(KC, g - k0)
                        gt = gp1.tile([P, KC, T1COLS], BF, tag="g1")
                        nc.gpsimd.dma_gather(
                            gt[:, 0:kn, :], ag1_out[:],
                            idx_sb[:, 8 * (int(goff[l]) + k0):
                                   8 * (int(goff[l]) + k0 + kn)],
                            P * kn, P * kn, T1COLS, single_packet=False,
                            queue_num=qrr[0] % NSWQ)
                        qrr[0] += 1
                        als_v = gt[:, 0:kn, 512:520].bitcast(F32)
                        u = wp.tile([P, KC, H1], F32, tag="u1")
                        nc.vector.tensor_tensor(
                            out=u[:, 0:kn, :], in0=als_v,
                            in1=_bap(ald1[:, l, :], [(0, kn), (1, H1)]),
                            op=OP.add)
                        lk = wp.tile([P, KC, H1], F32, tag="lk1")
                        nc.scalar.activation(lk[:, 0:kn, :], u[:, 0:kn, :],
                                             AF.Prelu, alpha=0.2)
                        exb = wp.tile([P, KC, H1], BF, tag="exb1")
                        nc.scalar.activation(exb[:, 0:kn, :], lk[:, 0:kn, :],
                                             AF.Exp)
                        dt_ = wp.tile([P, H1], F32, tag="dt1")
                        red = dt_ if k0 else den
                        nc.vector.tensor_reduce(
                            out=red[:], in_=_bap(exb[:], [(1, H1), (H1, kn)]),
                            axis=mybir.AxisListType.X, op=OP.add)
                        if k0:
                            dn2 = wp.tile([P, H1], F32, tag="dn1b")
                            nc.vector.tensor_add(dn2[:], den[:], dt_[:])
                            den = dn2
                        wt = wtp.tile([P, KC, 512], BF, tag="w1")
                        nc.vector.tensor_tensor(
                            out=wt[:, 0:kn, :],
                            in0=_bap(gt[:], [(T1COLS, kn), (HID, H1),
                                             (1, HID)]),
                            in1=_bap(exb[:], [(H1, kn), (1, H1), (0, HID)]),
                            op=OP.mult)
                        for k in range(kn):
                            nc.tensor.matmul(psA[:], lhsT=ident[:],
                                             rhs=wt[:, k, :],
                                             start=(k0 + k == 0),
                                             stop=(k0 + k == g - 1))
                        k0 += kn
                    dne = wp.tile([P, H1], F32, tag="dne1")
                    nc.vector.tensor_scalar_add(out=dne[:], in0=den[:],
                                                scalar1=1e-30)
                    denr = wp.tile([P, H1], F32, tag="dr1")
                    nc.vector.reciprocal(denr[:], dne[:])
                    nc.vector.tensor_tensor(
                        out=h5[:, i, :], in0=psA[:],
                        in1=_bap(denr[:], [(1, H1), (0, HID)]), op=OP.mult)
                # chunk-batched LN: one wide op per step instead of 5 narrow
                if "bias1" not in triv:
                    nc.vector.tensor_tensor(
                        out=h5[:], in0=h5[:],
                        in1=_bap(cc[:, CC_BIAS1:CC_BIAS1 + 512],
                                 [(0, CB), (1, 512)]), op=OP.add)
                mu5 = wp.tile([P, CB], F32, tag="mu1")
                nc.vector.tensor_reduce(
                    out=mu5[:], in_=_bap(h5[:], [(512, CB), (1, 512)]),
                    axis=mybir.AxisListType.X, op=OP.add)
                nc.scalar.mul(mu5[:], mu5[:], 1.0 / 512)
                nc.vector.tensor_tensor(
                    out=h5[:], in0=h5[:],
                    in1=_bap(mu5[:], [(1, CB), (0, 512)]), op=OP.subtract)
                for i in range(CB):
                    nc.scalar.activation(sqdump[:], h5[:, i, :], AF.Square,
                                         accum_out=vs[:, i:i + 1])
                rst = rstd_of(vs, 1.0 / 512, "1")
                nc.vector.tensor_tensor(
                    out=h5[:], in0=h5[:],
                    in1=_bap(rst[:], [(1, CB), (0, 512)]), op=OP.mult)
                if "g1b1" not in triv:
                    nc.vector.tensor_tensor(
                        out=h5[:], in0=h5[:],
                        in1=_bap(cc[:, CC_G1:CC_G1 + 512],
                                 [(0, CB), (1, 512)]), op=OP.mult)
                    nc.vector.tensor_tensor(
                        out=h5[:], in0=h5[:],
                        in1=_bap(cc[:, CC_B1:CC_B1 + 512],
                                 [(0, CB), (1, 512)]), op=OP.add)
                h1b5 = hb.tile([P, CB, 512], BF, tag="h1b5")
                nc.scalar.activation(h1b5[:], h5[:], AF.Gelu)
                for i in range(CB):
                    l = order[j * CB + i]
                    ps3 = pss.tile([P, 130], F32, tag="mm2")
                    for cch in range(4):
                        hT = wp.tile([P, P], BF, tag="hT")
                        transpose_to(hT[:], h1b5[:, i, cch * P:(cch + 1) * P])
                        nc.tensor.matmul(ps3[:], lhsT=hT[:], rhs=w2s[:, cch, :],
                                         start=(cch == 0), stop=(cch == 3))
                    t2 = wp.tile([P, T2USED], BF, tag="t2")
                    nc.vector.tensor_copy(out=t2[:, 0:128], in_=ps3[:, 0:128])
                    nc.vector.tensor_tensor(
                        out=t2[:, 128:130].bitcast(F32), in0=ps3[:, 128:129],
                        in1=afix[:, l, 4:5], op=OP.add)
                    nc.vector.tensor_copy(out=ald2[:, l, :], in_=ps3[:, 129:130])
                    nc.sync.dma_start(
                        ag2_in[l * P:(l + 1) * P, 0:T2USED], t2[:])
            nc.gpsimd.collective_compute(
                "AllGather", OP.bypass, replica_groups=rg,
                ins=[ag2_in[:].opt()], outs=[ag2_out[:].opt()])

            # ================= phase 4: GAT layer 2 ==========================
            # single head: leaky(als+ald) folds into ACT Prelu (per-partition
            # ald2 bias), exp's accum_out yields the softmax denominator
            for j in range(NAG):
                vs = vsp.tile([P, CB], F32, tag="vs2")
                h5b = hb.tile([P, CB, 128], F32, tag="h5b")
                for i in range(CB):
                    l = order[j * CB + i]
                    g = ghat[l]
                    psB_t = ps.tile([P, 512], F32, tag="big")
                    psB = psB_t[:, 0:128]
                    den = None
                    k0 = 0
                    while k0 < g:
                        kn = min(KC, g - k0)
                        gt = gp2.tile([P, KC, T2COLS], BF, tag="g2")
                        nc.gpsimd.dma_gather(
                            gt[:, 0:kn, :], ag2_out[:],
                            idx_sb[:, 8 * (int(goff[l]) + k0):
                                   8 * (int(goff[l]) + k0 + kn)],
                            P * kn, P * kn, T2COLS, single_packet=False,
                            queue_num=qrr[0] % NSWQ)
                        qrr[0] += 1
                        als_v = gt[:, 0:kn, 128:130].bitcast(F32)
                        lk = wp.tile([P, KC, 1], F32, tag="lk2")
                        nc.scalar.activation(lk[:, 0:kn, :], als_v,
                                             AF.Prelu, bias=ald2[:, l, :],
                                             alpha=0.2)
                        exb = wp.tile([P, KC, 1], BF, tag="exb2")
                        dt_ = wp.tile([P, 1], F32, tag="dt2")
                        nc.scalar.activation(exb[:, 0:kn, :], lk[:, 0:kn, :],
                                             AF.Exp, accum_out=dt_[:])
                        if k0:
                            dn2 = wp.tile([P, 1], F32, tag="dn2b")
                            nc.vector.tensor_add(dn2[:], den[:], dt_[:])
                            den = dn2
                        else:
                            den = dt_
                        wt = wt2p.tile([P, KC, 128], BF, tag="w2")
                        nc.vector.tensor_tensor(
                            out=wt[:, 0:kn, :],
                            in0=_bap(gt[:], [(T2COLS, kn), (1, 128)]),
                            in1=_bap(exb[:], [(1, kn), (0, 128)]),
                            op=OP.mult)
                        for k in range(kn):
                            nc.tensor.matmul(psB[:], lhsT=ident[:],
                                             rhs=wt[:, k, :],
                                             start=(k0 + k == 0),
                                             stop=(k0 + k == g - 1))
                        k0 += kn
                    dne = wp.tile([P, 1], F32, tag="dne2")
                    nc.vector.tensor_scalar_add(out=dne[:], in0=den[:],
                                                scalar1=1e-30)
                    denr = wp.tile([P, 1], F32, tag="dr2")
                    nc.vector.reciprocal(denr[:], dne[:])
                    nc.vector.tensor_scalar_mul(out=h5b[:, i, :], in0=psB[:],
                                                scalar1=denr[:])
                if "bias2" not in triv:
                    nc.vector.tensor_tensor(
                        out=h5b[:], in0=h5b[:],
                        in1=_bap(cc[:, CC_BIAS2:CC_BIAS2 + 128],
                                 [(0, CB), (1, 128)]), op=OP.add)
                mu5 = wp.tile([P, CB], F32, tag="mu2")
                nc.vector.tensor_reduce(
                    out=mu5[:], in_=_bap(h5b[:], [(128, CB), (1, 128)]),
                    axis=mybir.AxisListType.X, op=OP.add)
                nc.scalar.mul(mu5[:], mu5[:], 1.0 / 128)
                nc.vector.tensor_tensor(
                    out=h5b[:], in0=h5b[:],
                    in1=_bap(mu5[:], [(1, CB), (0, 128)]), op=OP.subtract)
                for i in range(CB):
                    nc.scalar.activation(sqdump[:, 0:128], h5b[:, i, :],
                                         AF.Square, accum_out=vs[:, i:i + 1])
                rst = rstd_of(vs, 1.0 / 128, "2")
                nc.vector.tensor_tensor(
                    out=h5b[:], in0=h5b[:],
                    in1=_bap(rst[:], [(1, CB), (0, 128)]), op=OP.mult)
                if "g2b2" not in triv:
                    nc.vector.tensor_tensor(
                        out=h5b[:], in0=h5b[:],
                        in1=_bap(cc[:, CC_G2:CC_G2 + 128],
                                 [(0, CB), (1, 128)]), op=OP.mult)
                    nc.vector.tensor_tensor(
                        out=h5b[:], in0=h5b[:],
                        in1=_bap(cc[:, CC_B2:CC_B2 + 128],
                                 [(0, CB), (1, 128)]), op=OP.add)
                nc.scalar.activation(h2b_all[:, j * CB:(j + 1) * CB, :],
                                     h5b[:], AF.Gelu)
                for i in range(CB):
                    pos = j * CB + i
                    l = order[pos]
                    hoT = wp.tile([P, P], BF, tag="hoT")
                    transpose_to(hoT[:], h2b_all[:, pos, :])
                    pso_t = pss.tile([P, 130], F32, tag="mm2")
                    pso = pso_t[:, 0:D_OUT]
                    nc.tensor.matmul(pso[:], lhsT=hoT[:], rhs=wos[:],
                                     start=True, stop=True)
                    if "bo" in triv:
                        z = pso
                    else:
                        z = wp.tile([P, D_OUT], F32, tag="z")
                        nc.vector.tensor_tensor(out=z[:], in0=pso[:],
                                                in1=cc[:, CC_BO:CC_BO + 32],
                                                op=OP.add)
                    m = wp.tile([P, 1], F32, tag="zm")
                    nc.vector.tensor_reduce(out=m[:], in_=z[:],
                                            axis=mybir.AxisListType.X,
                                            op=OP.max)
                    nc.vector.tensor_scalar_sub(out=zc_all[:, l, :], in0=z[:],
                                                scalar1=m[:])
                    ez = wp.tile([P, D_OUT], F32, tag="ez")
                    nc.scalar.activation(ez[:], zc_all[:, l, :], AF.Exp,
                                         accum_out=sden_all[:, l:l + 1])

            # ================= log_softmax tail ==============================
            for l in order:
                lnd = wp.tile([P, 1], F32, tag="lnd")  # noqa: order-iter
                nc.scalar.activation(lnd[:], sden_all[:, l:l + 1], AF.Ln)
                res = wp.tile([P, D_OUT], F32, tag="res")
                nc.vector.tensor_scalar_sub(out=res[:], in0=zc_all[:, l, :],
                                            scalar1=lnd[:])
                nc.sync.dma_start(out[l * P:(l + 1) * P, :], res[:])

    nc.compile()
    return nc


_CACHE = {}
_LAST_RUN = {}


def kernel(x, edge_index, g_in, b_in, W1, att1_s, att1_d, bias1, g1, b1,
           W2, att2_s, att2_d, bias2, g2, b2, Wo, bo):
    prep = prepare_inputs(x, edge_index)
    wts = prepare_weights(W1, att1_s, att1_d, bias1, g1, b1, g_in, b_in,
                          W2, att2_s, att2_d, bias2, g2, b2, Wo, bo)

    triv = []
    if np.allclose(g_in, 1) and np.allclose(b_in, 0):
        triv.append("gb0")
    if np.allclose(g1, 1) and np.allclose(b1, 0):
        triv.append("g1b1")
    if np.allclose(g2, 1) and np.allclose(b2, 0):
        triv.append("g2b2")
    if np.allclose(bias1, 0):
        triv.append("bias1")
    if np.allclose(bias2, 0):
        triv.append("bias2")
    if np.allclose(bo, 0):
        triv.append("bo")
    triv = tuple(sorted(triv))

    key = (tuple(prep["ghat"]), triv)
    if key not in _CACHE:
        _CACHE[key] = build_program(prep["ghat"], triv=triv)
    nc = _CACHE[key]

    in_maps = []
    for c in range(NCORES):
        in_maps.append({
            "x_own": prep["x_own"][c],
            "idxw": prep["idxw"][c],
            "alsfix": prep["alsfix"][c],
            "w1ext": wts["w1ext"],
            "w2ext": wts["w2ext"].astype(BF16),
            "woext": wts["woext"],
            "colconst": wts["colconst"],
        })

    _LAST_RUN.update(nc=nc, in_maps=in_maps, prep=prep)
    res = bass_utils.run_bass_kernel_spmd(nc, in_maps,
                                          core_ids=list(range(NCORES)))
    outs = [res.results[c]["out"] for c in range(NCORES)]

    newid = prep["newid"]
    blk = newid // P
    core = blk % NCORES
    row = (blk // NCORES) * P + newid % P
    full = np.empty((N, D_OUT), dtype=np.float32)
    for c in range(NCORES):
        sel = core == c
        full[sel] = outs[c][row[sel]]
    return full
